# revision 15
# baseline (speedup 1.0000x reference)
"""Multi-head attention (16 heads, head_dim 64, B=2, S=2048) on 8 trn2 cores.

Sharding: tensor-parallel over heads — core i computes heads 2i, 2i+1 for both
batch elements. Each core receives the full X^T [1024, 4096] in bf16 plus its
128-row slice of Wq/Wk/Wv (bf16, transposed to [1024, 128] lhsT layout), and
returns ctx^T [128, 4096] f32, already softmax-normalized.

v4 design (PE/ACT co-bottleneck; measured 195.6us on HW at full clock):
  All matmul operands bf16, psum f32. Iteration space = (b, qh[0..3], kc[0..15])
  with QG=512 q-blocks and both heads per iteration:
    scores h0 -> scp[:, 0:512]   (PE row-tile (0,0): kt rows 0:64)
    scores h1 -> scp[:, 512:1024] (PE row-tile (64,0): kt rows 64:128)
  The two scores matmuls contract over head_dim=64 only, so they occupy
  disjoint PE row halves and run concurrently (~2x scores throughput vs v3).
  One exp per iteration, N=1024 covering both heads (ACT floor 147us).
  ctx: per head M=65 (ones column -> softmax denom), N=512, accumulated in
  cx[65,512] psum over 16 kc, drained LAG iterations behind exp.
  PSUM: psA 2x[128,1024] (4) + cx 2x[65,512] (2) + psPq (1) + psPv (1) = 8.
  Phase-1 (proj+RoPE+V-transpose) woven into the attention stream on
  per-piece due-iteration schedules; warm matmul bursts hold the PE p-state
  up through the DMA-bound head.
"""
import collections
import os
import sys

for _p in ("/opt/trn_rl_repo", "/root/.axon_site/_ro/trn_rl_repo"):
    if os.path.isdir(_p) and _p not in sys.path:
        sys.path.insert(0, _p)

import numpy as np
import ml_dtypes

import concourse.bass as bass  # noqa: F401
import concourse.mybir as mybir
import concourse.tile as tile
from concourse import bacc
from concourse.bass_utils import run_bass_kernel_spmd

dt = mybir.dt
BF16 = ml_dtypes.bfloat16

B, S, NH, HD = 2, 2048, 16, 64
H = NH * HD            # 1024
T = B * S              # 4096
NCORES = 8
HPC = NH // NCORES     # heads per core = 2
DPC = HPC * HD         # dims per core = 128
CHUNK = 512            # token chunk for QKV projection
NCHUNK = 8
KC = 128               # k-token chunk in attention
NKC = S // KC          # 16 per batch
QG = 512               # q extent per attention group
NQG = S // QG          # 4 q-blocks per batch
VW = HD + 1            # 65: 64 dims then the ones column (softmax denom)
KTILES = H // 128      # 8
LAG = 12               # ctx matmul lag (iterations) behind scores/exp
NITER = B * NQG * NKC  # 128

_prog_cache = {}
_last_in_maps = None


def _build_program():
    nc = bacc.Bacc("TRN2", target_bir_lowering=False, debug=False,
                   num_devices=NCORES)
    f32 = dt.float32
    bf = dt.bfloat16

    xt_d = nc.declare_dram_parameter("xt", [NCHUNK, 128, KTILES, CHUNK], bf,
                                     isOutput=False)
    wq_d = nc.declare_dram_parameter("wq", [128, KTILES, DPC], bf,
                                     isOutput=False)
    wk_d = nc.declare_dram_parameter("wk", [128, KTILES, DPC], bf,
                                     isOutput=False)
    wv_d = nc.declare_dram_parameter("wv", [128, KTILES, DPC], bf,
                                     isOutput=False)
    cos_d = nc.declare_dram_parameter("cos2", [DPC, S], bf, isOutput=False)
    sin_d = nc.declare_dram_parameter("sins", [DPC, S], bf, isOutput=False)
    ident_d = nc.declare_dram_parameter("ident", [128, 128], bf,
                                        isOutput=False)
    ctxt_d = nc.declare_dram_parameter("ctxt", [DPC, T], f32, isOutput=True)

    Exp = mybir.ActivationFunctionType.Exp

    with tile.TileContext(nc) as tc:
        with (
            tc.tile_pool(name="persist", bufs=1) as pp,
            tc.tile_pool(name="consts", bufs=1) as cp,
            tc.tile_pool(name="p1", bufs=5) as p1,
            tc.tile_pool(name="p1s", bufs=3) as p1s,
            tc.tile_pool(name="p2e", bufs=LAG + 4) as p2e,
            tc.tile_pool(name="p2o", bufs=2) as p2o,
            # PSUM: scores 2x2 banks + qk-proj 1 + v-proj/transpose 1
            #       + ctx 2x1 = 8
            tc.tile_pool(name="psA", bufs=2, space="PSUM") as psA,
            tc.tile_pool(name="psPq", bufs=1, space="PSUM") as psPq,
            tc.tile_pool(name="psPv", bufs=1, space="PSUM") as psPv,
            tc.tile_pool(name="psCX", bufs=1, space="PSUM") as psCX,
        ):
            qt = pp.tile([DPC, T], bf, tag="qt")
            kt = pp.tile([DPC, T], bf, tag="kt")
            vaug = pp.tile([128, T // 128, HPC * VW], bf, tag="vaug")

            cos_sb = cp.tile([DPC, S], bf, tag="cos")
            sin_sb = cp.tile([DPC, S], bf, tag="sin")
            ident = cp.tile([128, 128], bf, tag="ident")
            wq_sb = cp.tile([128, KTILES, DPC], bf, tag="wq")
            wk_sb = cp.tile([128, KTILES, DPC], bf, tag="wk")
            wv_sb = cp.tile([128, KTILES, DPC], bf, tag="wv")

            warm = cp.tile([128, 512], bf, tag="warm")
            nc.vector.memset(warm[:], 0.25)

            def warm_burst(n=2):
                wps = psA.tile([128, 2 * QG], f32, tag="A", name="wps")
                for i in range(n):
                    nc.tensor.matmul(wps[:, 0:512], warm[:, 0:128],
                                     warm[:], start=True, stop=True)

            xt_tiles = {}

            def load_chunk(c):
                if c in xt_tiles or c >= NCHUNK:
                    return
                xt_t = p1.tile([128, KTILES, CHUNK], bf, tag="xt")
                half = KTILES // 2
                nc.sync.dma_start(out=xt_t[:, 0:half, :],
                                  in_=xt_d[c, :, 0:half, :])
                nc.sync.dma_start(out=xt_t[:, half:KTILES, :],
                                  in_=xt_d[c, :, half:KTILES, :])
                xt_tiles[c] = xt_t

            # DMA issue order = first-use order; warm bursts into the idle
            # psA pool keep the PE p-state up through the DMA-bound head.
            warm_burst(4)
            nc.sync.dma_start(out=wq_sb[:], in_=wq_d[:])
            nc.sync.dma_start(out=cos_sb[:], in_=cos_d[:])
            warm_burst(2)
            nc.sync.dma_start(out=sin_sb[:], in_=sin_d[:])
            load_chunk(0)
            warm_burst(2)
            nc.sync.dma_start(out=wk_sb[:], in_=wk_d[:])
            load_chunk(1)
            warm_burst(2)
            nc.sync.dma_start(out=ident[:], in_=ident_d[:])
            nc.sync.dma_start(out=wv_sb[:], in_=wv_d[:])

            ones_sb = cp.tile([128, T // 128], bf, tag="ones")
            nc.vector.memset(ones_sb[:], 1.0)
            for h in range(HPC):
                nc.vector.tensor_copy(
                    vaug[:, :, h * VW + HD:h * VW + HD + 1], ones_sb[:])

            def proj_pieces(c, which, warm_fill=False):
                """Projection + RoPE for q or k of chunk c (2-MM pieces)."""
                load_chunk(c)
                xt_t = xt_tiles[c]
                pos = (c * CHUNK) % S
                cs = cos_sb[:, pos:pos + CHUNK]
                sn = sin_sb[:, pos:pos + CHUNK]
                w_sb, dst = ((wq_sb, qt) if which == "q" else (wk_sb, kt))
                ps = psPq.tile([DPC, CHUNK], f32, tag="Pq")
                for k0 in range(0, KTILES, 2):
                    for k in range(k0, k0 + 2):
                        nc.tensor.matmul(
                            ps[:], w_sb[:, k, :], xt_t[:, k, :],
                            start=(k == 0), stop=(k == KTILES - 1))
                    if warm_fill:
                        warm_burst(1)
                    yield
                raw = p1s.tile([DPC, CHUNK], bf, tag="raw")
                nc.vector.tensor_copy(raw[:], ps[:])
                t1 = p1s.tile([DPC, CHUNK], bf, tag="t1")
                nc.vector.tensor_mul(t1[:], raw[:], cs)
                rot = p1s.tile([DPC, CHUNK], bf, tag="rot")
                hh = HD // 2
                for blk in range(DPC // hh):
                    sb = ((blk // 2) * 2) + (1 - blk % 2)
                    nc.sync.dma_start(
                        out=rot[blk * hh:(blk + 1) * hh, :],
                        in_=raw[sb * hh:(sb + 1) * hh, :])
                if warm_fill:
                    warm_burst(2)
                yield
                t2 = p1s.tile([DPC, CHUNK], bf, tag="t2")
                nc.vector.tensor_mul(t2[:], rot[:], sn)
                nc.vector.tensor_add(
                    dst[:, c * CHUNK:(c + 1) * CHUNK], t1[:], t2[:])
                yield

            def v_pieces(c):
                """V projection + transpose into vaug for chunk c."""
                load_chunk(c)
                xt_t = xt_tiles[c]
                psv = psPv.tile([DPC, CHUNK], f32, tag="Pv")
                for k0 in range(0, KTILES, 2):
                    for k in range(k0, k0 + 2):
                        nc.tensor.matmul(
                            psv[:], wv_sb[:, k, :], xt_t[:, k, :],
                            start=(k == 0), stop=(k == KTILES - 1))
                    yield
                vt = p1s.tile([DPC, CHUNK], bf, tag="vt")
                nc.vector.tensor_copy(vt[:], psv[:])
                yield
                for j in range(CHUNK // 128):
                    tp = psPv.tile([128, 128], bf, tag="Pv")
                    nc.tensor.transpose(tp[:], vt[:, j * 128:(j + 1) * 128],
                                        ident[:])
                    tt = c * (CHUNK // 128) + j
                    for h in range(HPC):
                        nc.vector.tensor_copy(
                            vaug[:, tt, h * VW:h * VW + HD],
                            tp[:, h * HD:(h + 1) * HD])
                    yield

            # ---------- attention pipeline ----------
            def emit_scores(b, qh, kc):
                """Both heads' scores for one (b, q-block, k-chunk):
                row-tiled pair + one exp N=1024 covering both heads."""
                q0 = b * S + qh * QG
                k0 = b * S + kc * KC
                scp = psA.tile([128, 2 * QG], f32, tag="A")
                nc.tensor.matmul(scp[:, 0:QG], kt[0:HD, k0:k0 + KC],
                                 qt[0:HD, q0:q0 + QG], start=True, stop=True)
                nc.tensor.matmul(scp[:, QG:2 * QG], kt[HD:DPC, k0:k0 + KC],
                                 qt[HD:DPC, q0:q0 + QG], start=True, stop=True)
                e = p2e.tile([128, 2 * QG], bf, tag="e")
                nc.scalar.activation(e[:], scp[:], Exp, scale=0.125)
                return e

            cx_state = {}

            def emit_ctx(b, qh, kc, e):
                if kc == 0:
                    cx_state["h0"] = psCX.tile([VW, QG], f32, tag="cx0",
                                               name="cx0")
                    cx_state["h1"] = psCX.tile([VW, QG], f32, tag="cx1",
                                               name="cx1")
                tt = (b * S) // 128 + kc
                for h in range(HPC):
                    cx = cx_state["h0" if h == 0 else "h1"]
                    nc.tensor.matmul(
                        cx[:], vaug[:, tt, h * VW:(h + 1) * VW],
                        e[:, h * QG:(h + 1) * QG],
                        start=(kc == 0), stop=(kc == NKC - 1))
                if kc == NKC - 1:
                    q0 = b * S + qh * QG
                    for h in range(HPC):
                        cx = cx_state["h0" if h == 0 else "h1"]
                        ctxu = p2o.tile([VW, QG], f32, tag="ctxu%d" % h)
                        nc.vector.tensor_copy(ctxu[:], cx[:])
                        rsq = p2o.tile([128, QG // 128], f32,
                                       tag="rsq%d" % h)
                        nc.sync.dma_start(out=rsq[:], in_=ctxu[HD:VW, :])
                        rsqi = p2o.tile([128, QG // 128], f32,
                                        tag="rsqi%d" % h)
                        nc.vector.reciprocal(rsqi[:], rsq[:])
                        r0 = p2o.tile([1, QG], f32, tag="r0%d" % h)
                        nc.sync.dma_start(out=r0[:], in_=rsqi[:])
                        rb = p2o.tile([HD, QG], f32, tag="rb%d" % h)
                        nc.gpsimd.partition_broadcast(rb[:], r0[:])
                        oc = p2o.tile([HD, QG], f32, tag="oc%d" % h)
                        nc.vector.tensor_mul(oc[:], ctxu[0:HD, :], rb[:])
                        nc.sync.dma_start(
                            out=ctxt_d[h * HD:(h + 1) * HD, q0:q0 + QG],
                            in_=oc[:])

            # head: q,k of chunks 0/1 (with warm fill). c2/c3 DMAs issue
            # early so woven pieces never wait on HBM.
            for which in ("q", "k"):
                for _ in proj_pieces(0, which, warm_fill=True):
                    pass
            load_chunk(2)
            for _ in proj_pieces(1, "k", warm_fill=True):
                pass
            load_chunk(3)
            load_chunk(4)

            # phase-1 weave: per-generator linear due-iteration schedules
            weave = collections.deque()

            def add(due_lo, due_hi, gen, n_hint, deadline=None):
                # spread dues linearly over [due_lo, due_hi]; the final
                # piece fires during pump(due_hi), which precedes iteration
                # due_hi+1 — so require due_hi < deadline.
                if deadline is not None:
                    assert due_hi < deadline, (due_lo, due_hi, deadline)
                weave.append([due_lo, due_hi, gen, n_hint, 0])

            # psPq lane (strictly disjoint windows):
            add(0, 4, proj_pieces(2, "k"), 6, deadline=8)     # kt c2 due 8
            add(5, 9, proj_pieces(3, "k"), 6, deadline=12)    # kt c3 due 12
            add(10, 14, proj_pieces(1, "q"), 6, deadline=16)  # qt c1 due 16
            add(15, 26, proj_pieces(2, "q"), 6, deadline=32)  # qt c2 due 32
            add(27, 38, proj_pieces(3, "q"), 6, deadline=48)  # qt c3 due 48
            add(39, 50, proj_pieces(4, "q"), 6, deadline=64)  # qt c4 due 64
            add(51, 58, proj_pieces(4, "k"), 6, deadline=64)  # kt c4 due 64
            add(59, 64, proj_pieces(5, "k"), 6, deadline=68)  # kt c5 due 68
            add(65, 69, proj_pieces(6, "k"), 6, deadline=72)  # kt c6 due 72
            add(70, 74, proj_pieces(7, "k"), 6, deadline=76)  # kt c7 due 76
            add(75, 79, proj_pieces(5, "q"), 6, deadline=80)  # qt c5 due 80
            add(80, 93, proj_pieces(6, "q"), 6, deadline=96)  # qt c6 due 96
            add(94, 110, proj_pieces(7, "q"), 6, deadline=112)  # qt c7 due 112
            # psPv lane (disjoint windows); vaug chunk c's tiles are read at
            # iters 4c+LAG .. 4c+LAG+3 (b=0), 64+4(c-4)+LAG .. +3 (b=1)
            add(0, 10, v_pieces(0), 9, deadline=15)    # read 12-15
            add(11, 15, v_pieces(1), 9, deadline=19)   # read 16-19
            add(16, 19, v_pieces(2), 9, deadline=23)   # read 20-23
            add(20, 24, v_pieces(3), 9, deadline=27)   # read 24-27
            add(25, 45, v_pieces(4), 9, deadline=79)   # read 76-79
            add(46, 62, v_pieces(5), 9, deadline=83)   # read 80-83
            add(63, 74, v_pieces(6), 9, deadline=87)   # read 84-87
            add(75, 84, v_pieces(7), 9, deadline=91)   # read 88-91

            def pump(i):
                # advance every weave entry whose linear schedule is due
                for entry in list(weave):
                    lo, hi, gen, n, done = entry
                    while True:
                        due = lo + (hi - lo) * entry[4] / max(n, 1)
                        if due > i:
                            break
                        try:
                            next(gen)
                            entry[4] += 1
                        except StopIteration:
                            weave.remove(entry)
                            break

            iters = []
            for b in (0, 1):
                for qh in range(NQG):
                    for kc in range(NKC):
                        iters.append((b, qh, kc))

            pending = collections.deque()
            for i, (b, qh, kc) in enumerate(iters):
                # scores first: they gate the exp stream; woven pieces go
                # after so they fill the PE slack instead of delaying exp.
                e = emit_scores(b, qh, kc)
                pending.append((b, qh, kc, e))
                if len(pending) > LAG:
                    emit_ctx(*pending.popleft())
                # drain the lag early near the end so the tail is short
                if i >= len(iters) - LAG and pending:
                    emit_ctx(*pending.popleft())
                pump(i)
                if i == 20:
                    load_chunk(5)
                if i == 35:
                    load_chunk(6)
                if i == 50:
                    load_chunk(7)
            while weave:
                lo, hi, gen, n, done = weave[0]
                try:
                    next(gen)
                except StopIteration:
                    weave.popleft()
            while pending:
                emit_ctx(*pending.popleft())

    nc.compile()
    return nc


def _host_tables():
    inv_freq = 1.0 / (10000.0 ** (np.arange(0, HD, 2, dtype=np.float32) / HD))
    t = np.arange(S, dtype=np.float32)
    freqs = np.outer(t, inv_freq)            # [S, 32]
    emb = np.concatenate([freqs, freqs], axis=-1)  # [S, 64]
    cosT = np.cos(emb).T.astype(np.float32)  # [64, S]
    sinT = np.sin(emb).T.astype(np.float32)
    sin_signed = sinT.copy()
    sin_signed[:HD // 2] *= -1.0             # rows d<32 multiply -sin
    cos2 = np.ascontiguousarray(np.vstack([cosT, cosT])).astype(BF16)
    sins = np.ascontiguousarray(np.vstack([sin_signed, sin_signed])).astype(BF16)
    ident = np.eye(128, dtype=np.float32).astype(BF16)
    return cos2, sins, ident


def kernel(hidden_states: np.ndarray, Wq: np.ndarray, Wk: np.ndarray,
           Wv: np.ndarray) -> np.ndarray:
    hidden_states = np.asarray(hidden_states, dtype=np.float32)
    Wq = np.asarray(Wq, dtype=np.float32)
    Wk = np.asarray(Wk, dtype=np.float32)
    Wv = np.asarray(Wv, dtype=np.float32)
    assert hidden_states.shape == (B, S, H), hidden_states.shape

    if "nc" not in _prog_cache:
        _prog_cache["nc"] = _build_program()
    nc = _prog_cache["nc"]

    xt = hidden_states.reshape(T, H).T  # [1024, 4096] view
    # pre-tile so each chunk DMA is contiguous per partition:
    # xt_tiled[c, p, k, t] = xt[k*128 + p, c*512 + t]
    xt_tiled = np.ascontiguousarray(
        xt.reshape(KTILES, 128, NCHUNK, CHUNK).transpose(2, 1, 0, 3)
    ).astype(BF16)
    cos2, sins, ident = _host_tables()

    def tile_w(W):
        # w_tiled[p, k, d] = W.T[k*128 + p, d]
        return np.ascontiguousarray(
            W.T.reshape(KTILES, 128, DPC).transpose(1, 0, 2)).astype(BF16)

    in_maps = []
    for i in range(NCORES):
        rows = slice(i * DPC, (i + 1) * DPC)
        in_maps.append({
            "xt": xt_tiled,
            "wq": tile_w(Wq[rows]),
            "wk": tile_w(Wk[rows]),
            "wv": tile_w(Wv[rows]),
            "cos2": cos2,
            "sins": sins,
            "ident": ident,
        })

    global _last_in_maps
    _last_in_maps = in_maps
    res = run_bass_kernel_spmd(nc, in_maps, list(range(NCORES)))

    # ctxt per core: [128 (2 heads x 64 dims), 4096 (2 batches x 2048)]
    full = np.stack([res.results[i]["ctxt"] for i in range(NCORES)])
    out = full.reshape(NCORES, HPC, HD, B, S).transpose(3, 4, 0, 1, 2)
    return np.ascontiguousarray(out.reshape(B, S, H), dtype=np.float32)
